# revision 1
# baseline (speedup 1.0000x reference)
"""GATv2-style attention layer on 8 Trainium2 NeuronCores (Bass/Tile SPMD).

Math (per head h):
    e[i,j]   = lrelu(ei[i] + ej[j]),  ei = x@W_h@a1, ej = x@W_h@a2
    att      = softmax_j(mask(e, adj))
    out      = mean_h(att @ h_feat)

Key transformation (removes all per-element transcendentals):
    exp(lrelu(z)) = exp(0.2 z) * max(exp(0.8 z), 1)
    with z = ei + ej, the exp(0.2 ei) factor cancels in the softmax, so the
    unnormalized score is
        pm[j,i] = adj[i,j] * max(Ti[i]*Sj[j], 1) * Vq[j]
    with Ti = exp(0.8 ei), Sj = exp(0.8 ej), Vq = exp(0.2 ej) -- all vectors.
    Vq folds into the matmul rhs (H~ = Vq * 0.25*h), so the [j,i] matrix work
    is exactly one tensor_scalar (mult + max-const, fp16 4x) and one
    tensor_tensor mask multiply (fp16 2x) per tile.

Sharding: core c owns destination rows i in [512c, 512c+512). W/x replicated
(h recomputed on every core -- cheaper than an all-gather), adj fed
pre-transposed+sliced per core so the score matrix is built directly in
[j_partition, i_free] layout for the PE aggregation matmul.
"""

import contextlib
import os
import sys

import numpy as np

for _p in ("/opt/trn_rl_repo", "/root/.axon_site/_ro/trn_rl_repo"):
    if os.path.isdir(_p) and _p not in sys.path:
        sys.path.append(_p)

import concourse.bass as bass
import concourse.mybir as mybir
from concourse import bacc
import concourse.tile as tile
from concourse.tile import add_dep_helper
from concourse.bass_utils import run_bass_kernel_spmd
from concourse.masks import make_identity

N = 4096
F_IN = 256
HEADS = 4
F_OUT = 64
CORES = 8
I_PER_CORE = N // CORES          # 512
P = 128
NJC = N // P                     # 32 j-chunks
NIC = I_PER_CORE // P            # 4 i-chunks
ICOL = F_OUT + 1                 # 65: [0.25*h | ones] per head
WCOLS = HEADS * F_OUT            # 256
KCH = 2                          # full 128-row K chunks of x (256) + 1 ones row

F32 = mybir.dt.float32
F16 = mybir.dt.float16

_BASS = None
LAST_RESULT = None


def _build(reps=1):
    nc = bacc.Bacc()
    xT_d = nc.dram_tensor("xT", [F_IN, N], F16, kind="ExternalInput")
    W_d = nc.dram_tensor("Wa", [F_IN, WCOLS], F16, kind="ExternalInput")
    tib_d = nc.dram_tensor("tib", [P, HEADS, I_PER_CORE], F16, kind="ExternalInput")
    ejr_d = nc.dram_tensor("ejr", [P, NJC, HEADS], F32, kind="ExternalInput")
    adjT_d = nc.dram_tensor("adjT", [N, I_PER_CORE], F16, kind="ExternalInput")
    out_d = nc.dram_tensor("out", [I_PER_CORE, F_OUT], F32, kind="ExternalOutput")

    EXP = mybir.ActivationFunctionType.Exp
    MULT = mybir.AluOpType.mult
    MAX = mybir.AluOpType.max
    ADD = mybir.AluOpType.add

    with tile.TileContext(nc) as tc:
        with (
            tc.tile_pool(name="cst", bufs=1) as cst,
            tc.tile_pool(name="adj", bufs=1) as adjp,
            tc.tile_pool(name="qpm", bufs=3) as qpm,
            tc.tile_pool(name="fin", bufs=2) as fin,
            tc.tile_pool(name="ps1", bufs=2, space="PSUM") as ps1,
            tc.tile_pool(name="psacc", bufs=1, space="PSUM") as psacc,
            (tc.For_i(0, reps, 1) if reps > 1 else contextlib.nullcontext()),
        ):
            # ---- loads -------------------------------------------------
            xk = [cst.tile([P, N], F16, name=f"xk{k}", tag=f"xk{k}")
                  for k in range(KCH)]
            wk = [cst.tile([P, WCOLS], F16, name=f"wk{k}", tag=f"wk{k}")
                  for k in range(KCH)]
            for k in range(KCH):
                for q_ in range(4):
                    nc.sync.dma_start(
                        xk[k][:, q_ * (N // 4):(q_ + 1) * (N // 4)],
                        xT_d[k * P:(k + 1) * P,
                             q_ * (N // 4):(q_ + 1) * (N // 4)])
                nc.sync.dma_start(wk[k][:], W_d[k * P:(k + 1) * P, :])
            tib = cst.tile([P, HEADS, I_PER_CORE], F16, tag="tib")
            nc.sync.dma_start(tib[:], tib_d[:])
            ejr = cst.tile([P, NJC, HEADS], F32, tag="ejr")
            nc.sync.dma_start(ejr[:], ejr_d[:])

            # ---- phase 1+2: rhs_h = [0.25*h_h fp16 | 1], node vectors --
            # Vq is folded into the mask STT, so rhs is just 0.25*h and a
            # constant ones column (s~ = sum_j pm since pm already has Vq).
            # rhs_h = [Vq_h * 0.25*h_h | Vq_h]: Vq applied during the PSUM
            # evacuation via ACT's free per-partition scale operand.
            rhs = cst.tile([P, NJC, HEADS, ICOL], F16, tag="rhs")
            sj = cst.tile([P, NJC, HEADS], F32, tag="sj")      # exp(0.8 ej)
            vq = cst.tile([P, NJC, HEADS], F32, tag="vq")      # exp(0.2 ej)
            nc.scalar.activation(sj[:], ejr[:], EXP, scale=0.8)
            nc.scalar.activation(vq[:], ejr[:], EXP, scale=0.2)
            for h in range(HEADS):
                nc.vector.tensor_copy(rhs[:, :, h, F_OUT], vq[:, :, h])
            CPY = mybir.ActivationFunctionType.Copy
            for j in range(NJC):
                pt = ps1.tile([P, WCOLS], F32, tag="ps_h")
                nc.tensor.matmul(pt[:], xk[0][:, j * P:(j + 1) * P], wk[0][:],
                                 start=True, stop=False)
                nc.tensor.matmul(pt[:], xk[1][:, j * P:(j + 1) * P], wk[1][:],
                                 start=False, stop=True)
                for h in range(HEADS):
                    nc.scalar.activation(rhs[:, j, h, :F_OUT],
                                         pt[:, h * F_OUT:(h + 1) * F_OUT],
                                         CPY, scale=vq[:, j, h:h + 1])

            # ---- phase 3: stream the score matrix ---------------------
            # adj slice fully resident (4 MiB); heads outer so each head's
            # accumulation owns the PSUM banks exclusively (start zeroes the
            # whole 2KB bank, so groups must not interleave within a bank).
            adj_sb = adjp.tile([P, NJC, I_PER_CORE], F16, tag="adj_sb")
            for j in range(NJC):
                nc.sync.dma_start(adj_sb[:, j, :], adjT_d[j * P:(j + 1) * P, :])

            # pre-touch tib (DMA) and sj (ACT) on DVE so the first
            # TensorScalarPtr needs at most one sync wait (HW encoding limit)
            junk = fin.tile([P, 2], F32, tag="junk")
            pt1 = nc.vector.tensor_copy(junk[:, 0:1], tib[:, 0, 0:1])
            pt2 = nc.vector.tensor_copy(junk[:, 1:2], sj[:, 0, 0:1])

            ident = cst.tile([P, P], F32, tag="ident")
            make_identity(nc, ident[:])

            ot = [fin.tile([P, F_OUT], F32, name=f"ot{ic}", tag=f"ot{ic}")
                  for ic in range(NIC)]
            # acc_h[f|s, i] accumulates [65, 512] per head; one group per bank
            acc = [psacc.tile([ICOL, I_PER_CORE], F32, name=f"acc{h}",
                              tag=f"acc{h}") for h in range(HEADS)]
            JG = 2   # j-chunks per batched mask multiply
            if True:
                for jg in range(NJC // JG):
                    q4 = qpm.tile([P, JG, HEADS, I_PER_CORE], F16, tag="q4")
                    pm4 = qpm.tile([P, JG, HEADS, I_PER_CORE], F16, tag="pm4")
                    for jj in range(JG):
                        j = jg * JG + jj
                        for h in range(HEADS):
                            qi = nc.vector.tensor_scalar(
                                q4[:, jj, h, :], tib[:, h, :],
                                sj[:, j, h:h + 1], 1.0, op0=MULT, op1=MAX)
                            if j == 0 and h == 0:
                                add_dep_helper(qi.ins, pt1.ins, sync=False,
                                               reason="pretouch order")
                                add_dep_helper(qi.ins, pt2.ins, sync=False,
                                               reason="pretouch order")
                    adj_b = adj_sb[:, jg * JG:(jg + 1) * JG, :].unsqueeze(
                        2).broadcast_to([P, JG, HEADS, I_PER_CORE])
                    nc.vector.tensor_tensor(pm4[:], q4[:], adj_b, op=MULT)
                    for jj in range(JG):
                        j = jg * JG + jj
                        for h in range(HEADS):
                            nc.tensor.matmul(acc[h][:], rhs[:, j, h, :],
                                             pm4[:, jj, h, :],
                                             start=(j == 0), stop=(j == NJC - 1))

            # finalize: evacuate, transpose back to [i, f], normalize, mean
            for h in range(HEADS):
                numt = fin.tile([ICOL, I_PER_CORE], F32, name=f"numt{h}",
                                tag="numt")
                nc.scalar.copy(numt[:], acc[h][:])
                for ic in range(NIC):
                    ps_t = ps1.tile([P, ICOL], F32, tag="ps_t")
                    nc.tensor.matmul(ps_t[:], numt[:, ic * P:(ic + 1) * P],
                                     ident[:ICOL, :ICOL], is_transpose=True,
                                     start=True, stop=True)
                    rec = fin.tile([P, 1], F32, tag="rec")
                    nc.vector.reciprocal(rec[:], ps_t[:, F_OUT:F_OUT + 1])
                    if h == 0:
                        nc.scalar.activation(ot[ic][:], ps_t[:, :F_OUT],
                                             CPY, scale=rec[:])
                    else:
                        nc.vector.scalar_tensor_tensor(
                            ot[ic][:], ps_t[:, :F_OUT], rec[:], ot[ic][:],
                            op0=MULT, op1=ADD)

            for ic in range(NIC):
                nc.sync.dma_start(out_d[ic * P:(ic + 1) * P, :], ot[ic][:])

    nc.finalize()
    return nc


def _host_prep(x, adj, W, a):
    x = np.asarray(x, np.float32)
    adj = np.asarray(adj)
    W = np.asarray(W, np.float32)
    a = np.asarray(a, np.float32).reshape(-1)
    a1, a2 = a[:F_OUT], a[F_OUT:]

    w1 = np.stack([W[:, 64 * h:64 * h + 64] @ a1 for h in range(HEADS)], 1)
    w2 = np.stack([W[:, 64 * h:64 * h + 64] @ a2 for h in range(HEADS)], 1)
    ei = x @ w1                                   # [N, H] f32
    ej = x @ w2                                   # [N, H] f32
    ti16 = np.exp(0.8 * ei).astype(np.float16)    # [N, H]

    xT = np.ascontiguousarray(x.T.astype(np.float16))
    Wa = np.empty((F_IN, WCOLS), np.float16)
    for h in range(HEADS):
        Wa[:, h * F_OUT:(h + 1) * F_OUT] = 0.25 * W[:, 64 * h:64 * h + 64]
    ejr = np.ascontiguousarray(ej.reshape(NJC, P, HEADS).transpose(1, 0, 2))
    adjT = adj.T.astype(np.float16)               # [j, i]

    in_maps = []
    for c in range(CORES):
        sl = slice(c * I_PER_CORE, (c + 1) * I_PER_CORE)
        in_maps.append({
            "xT": xT,
            "Wa": Wa,
            "tib": np.ascontiguousarray(
                np.broadcast_to(ti16[sl].T[None], (P, HEADS, I_PER_CORE))),
            "ejr": ejr,
            "adjT": np.ascontiguousarray(adjT[:, sl]),
        })
    return in_maps


def kernel(x, adj, W, a):
    global _BASS, LAST_RESULT
    if _BASS is None:
        _BASS = _build()
    in_maps = _host_prep(x, adj, W, a)
    res = run_bass_kernel_spmd(_BASS, in_maps, core_ids=list(range(CORES)))
    LAST_RESULT = res
    return np.concatenate([res.results[c]["out"] for c in range(CORES)], axis=0)

